# revision 83
# baseline (speedup 1.0000x reference)
"""Trainium2 Bass kernel for nn_Block_29832842838698 (nGPT-style transformer block).

B=2, T=2048, C=2048, H=16, D=128, SwiGLU FFN (8C fc -> split -> 4C proj).

Sharding over 8 NeuronCores:
  - QKV projections + attention: batch x head parallel. Core c handles batch
    c//4 and heads 4*(c%4)..+3 over the full causal T x T. Projection
    weights are pre-sliced per core on the host; two AllGathers (fp8 y)
    redistribute attention outputs back to token sharding.
  - Wo / residuals / MLP: token-parallel. Core c owns 512 tokens: batch0
    slice c (tokens 256c..256c+255 -> local cols 0..255) and batch1 slice
    7-c (-> cols 256..511), "zigzag" so the work stays balanced.
  - Activations are feature-major on-chip: [C(partitions), tokens(free)].

Precision strategy (nGPT: every branch is justnorm'd, so per-branch scale
factors cancel and lr~0.05 damps branch noise ~20x):
  - QKV projections + attention scores in bf16 (score logits are
    sqrt(D)-amplified -> keep 8-bit noise out of the logits).
  - exp(scores) (pT), v, y, Wo, FFN up and FFN down in fp8e4m3; the three
    big projections use MatmulPerfMode.DoubleRow (K=256 per matmul, ~1.7x
    tensor throughput, half the weight DMA). exp is shifted by EXP_BIAS so
    its max stays under fp8e4's 240 ceiling (softmax is shift-invariant);
    the denominator takes a +1e-6 bias so an all-underflow row divides by
    epsilon (y=0) instead of producing Inf. Fixed power-of-2 weight scales
    keep fp8 operands in the normal range; all scales cancel in justnorm.
  - Main residual chain + norm reductions in fp32/float32r. h2 (post-attn
    residual) is stored as 32*justnorm(.) so its fp8 cast for the FFN needs
    no extra scaling pass; the residual-2 constants absorb the 1/32.
"""

import os
import sys

sys.path.insert(0, "/opt/trn_rl_repo")

from contextlib import ExitStack

import numpy as np
import ml_dtypes

import concourse.bass as bass
import concourse.tile as tile
from concourse import mybir, bacc
from concourse.bass import ds
from concourse.bass_utils import run_bass_kernel_spmd

f32 = mybir.dt.float32
f32r = mybir.dt.float32r
bf16 = mybir.dt.bfloat16
f8 = mybir.dt.float8e4
f8e5 = mybir.dt.float8e5
AF = mybir.ActivationFunctionType
ALU = mybir.AluOpType
DR = mybir.MatmulPerfMode.DoubleRow

B, T, C, H, D = 2, 2048, 2048, 16, 128
NCORES = 8
TOK = 512            # tokens per core in the token-parallel phases
SL = 256             # slice length
KB = C // 128        # 16 feature blocks of C
K2 = KB // 2         # 8 feature block pairs (DoubleRow contraction)
JB = 4 * C // 128    # 64 blocks of the 4C ffn dim
JG = JB // 2         # 32 up-proj pair groups == down-proj block pairs
BASE_SCALE = 0.022097086912079608
SQK_MULT = 1.0 / BASE_SCALE
ALPHA_MULT = 0.05 / BASE_SCALE
SUV_MULT = C ** 0.5
SOFTMAX_SCALE = float(D) ** 0.5
H2_SCALE = 32.0      # h2 stored as 32*justnorm(.) for direct fp8 cast
WO_SCALE = 32.0      # Wo*32  -> fp8 operand std ~0.7 (cancels in justnorm)
WP_SCALE = 64.0      # Wproj*64 likewise
QKV_SCALE = 32.0     # Wq/Wk/Wv*32 likewise (justnorm'd q/k; y carries 32x)
EXP_BIAS5 = -0.5     # max logit sqrt(D)=11.31; e^10.81=49.4k < e5m2's 57344.
                     # e5m2 denormals reach 2^-16, so a whole softmax row can
                     # only underflow if every logit < -10.6 (cos < -0.94):
                     # impossible, unlike e4m3's 2^-9 floor.

DEBUG_TAPS = os.environ.get("KERNEL_DEBUG_TAPS", "")
PHASE_LEVEL = {"p1": 1, "p3": 2, "p45": 3, "all": 4}[
    os.environ.get("KERNEL_PHASES", "all")]
SIM_NO_CC = bool(os.environ.get("KERNEL_SIM_NO_CC", ""))


def _rope_colmap():
    """Head-wise column permutation: interleaved-pair rope -> rotate-half."""
    m = np.zeros(C, dtype=np.int64)
    for h in range(H):
        base = h * D
        for i in range(D // 2):
            m[base + i] = base + 2 * i
            m[base + 64 + i] = base + 2 * i + 1
    return m


def _build_program():
    nc = bacc.Bacc(None)
    dp = nc.declare_dram_parameter

    ext = {}
    ext["h_t"] = dp("h_t", [C, TOK], f32r, isOutput=False)
    ext["hb8_t"] = dp("hb8_t", [128, K2 * 2 * T], f8, isOutput=False)
    ext["cos_g"] = dp("cos_g", [D, T], bf16, isOutput=False)
    ext["sneg_g"] = dp("sneg_g", [D, T], bf16, isOutput=False)
    # pre-tiled weights (see _host_prep for layouts)
    ext["wq_my"] = dp("wq_my", [4 * 128, K2 * 2 * D], f8, isOutput=False)
    ext["wk_my"] = dp("wk_my", [4 * 128, K2 * 2 * D], f8, isOutput=False)
    ext["wv_my"] = dp("wv_my", [128, K2 * 2 * 4 * D], f8, isOutput=False)
    # fp8 DoubleRow layouts: [p, ..., j(2), cols], contraction pair j
    ext["wo8_t"] = dp("wo8_t", [128, 2 * K2 * 2 * 1024], f8, isOutput=False)
    ext["wfc8_t"] = dp("wfc8_t", [128, JG * K2 * 2 * 512], f8, isOutput=False)
    ext["wproj8_t"] = dp("wproj8_t", [128, 8 * 8 * 2 * 1024], f8,
                         isOutput=False)
    ext["sqk_my"] = dp("sqk_my", [D, 4], f32, isOutput=False)
    ext["lrs"] = dp("lrs", [128, 4 * KB + 3], f32, isOutput=False)
    ext["onesc"] = dp("onesc", [128, 128], f32r, isOutput=False)
    ext["onesb"] = dp("onesb", [128, 1], bf16, isOutput=False)
    ext["ones8e5"] = dp("ones8e5", [128, 2 * 16], f8e5, isOutput=False)
    ext["out_t"] = dp("out_t", [C, TOK], f32, isOutput=True)

    taps = {}
    for name, shape in [
        ("qhat", [4 * D, T]), ("khat", [4 * D, T]), ("vtok", [T, 4 * D]),
        ("ymine", [4 * D, T]), ("hatt", [C, TOK]), ("h2", [C, TOK]),
        ("hmlp", [C, TOK]),
    ]:
        if name in DEBUG_TAPS:
            taps[name] = dp("tap_" + name, shape, f32, isOutput=True)
    ext["taps"] = taps

    ext["y_mine1"] = nc.dram_tensor("y_mine1", [2 * D, T], f8)
    ext["y_all1"] = nc.dram_tensor("y_all1", [NCORES * 2 * D, T], f8,
                                   addr_space="Shared")
    ext["y_mine2"] = nc.dram_tensor("y_mine2", [2 * D, T], f8)
    ext["y_all2"] = nc.dram_tensor("y_all2", [NCORES * 2 * D, T], f8,
                                   addr_space="Shared")
    ext["RG"] = [list(range(NCORES))]

    with ExitStack() as ctx:
        ctx.enter_context(nc.allow_low_precision(
            reason="branch matmuls intentionally bf16/fp8; the nGPT justnorm "
                   "structure cancels scales and lr~0.05 damps branch noise; "
                   "main chain is fp32"))
        tc = ctx.enter_context(tile.TileContext(nc))
        _emit(ctx, tc, ext)
    nc.finalize()
    return nc


def _emit(ctx, tc, E):
    nc = tc.nc
    taps = E["taps"]
    RG = E["RG"]

    def allgather(mine, all_):
        if SIM_NO_CC:
            nc.sync.dma_start(out=all_[0:mine.shape[0], :], in_=mine[:])
        else:
            nc.gpsimd.collective_compute(
                "AllGather", ALU.bypass, replica_groups=RG,
                ins=[mine[:]], outs=[all_[:]])

    consts = ctx.enter_context(tc.tile_pool(name="consts", bufs=1))
    stat_sb = ctx.enter_context(tc.tile_pool(name="stat_sb", bufs=1))

    # ---------------- constants (scalar queue: tiny, ahead of bulk) --------
    ones_col = consts.tile([128, 1], f32r, tag="ones_col", name="ones_col")
    ones_row = consts.tile([1, 128], f32r, tag="ones_row", name="ones_row")
    ones_col_b = consts.tile([128, 1], bf16, tag="ones_col_b",
                             name="ones_col_b")
    # [128, 2, 16] so the DoubleRow lhsT slice [:, :, 0:1] has a 16B j-stride
    ones8e5 = consts.tile([128, 2, 16], f8e5, tag="ones8e5", name="ones8e5")
    nc.scalar.dma_start(out=ones_col[:], in_=E["onesc"][:, 0:1])
    nc.scalar.dma_start(out=ones_row[:], in_=E["onesc"][0:1, :])
    nc.scalar.dma_start(out=ones_col_b[:], in_=E["onesb"][:])
    nc.scalar.dma_start(out=ones8e5[:], in_=E["ones8e5"][:])
    sqk_t = consts.tile([D, 4], f32, tag="sqk", name="sqk")
    nc.scalar.dma_start(out=sqk_t[:], in_=E["sqk_my"][:])
    lrs = consts.tile([128, 4 * KB + 3], f32, tag="lrs", name="lrs")
    nc.scalar.dma_start(out=lrs[:], in_=E["lrs"][:])
    alr_t = lrs[:, 0 * KB:1 * KB]
    mlr_t = lrs[:, 1 * KB:2 * KB]
    alr1_t = lrs[:, 2 * KB:3 * KB]
    mlr1_t = lrs[:, 3 * KB:4 * KB]      # = (1 - lr_m) / H2_SCALE
    c132_t = lrs[:, 4 * KB:4 * KB + 1]  # = 1/32 (undo h28 scale in xm)
    ebias_t = lrs[:, 4 * KB + 1:4 * KB + 2]  # = EXP_BIAS5

    cbits = dict(ones_col=ones_col, ones_row=ones_row, stat_sb=stat_sb)

    # partition-id derived registers (used only for the y_all reads)
    pid = nc.sync.partition_id()
    PC_reg = nc.sync.snap(pid * SL, min_val=0, max_val=1792)
    PC1_reg = nc.sync.snap((7 - pid) * SL, min_val=0, max_val=1792)

    def stats_from_psum(nsq_ps, tagbase, scale=1.0):
        # shared tags: h/a/m stats are strictly sequential consumers.
        # Abs_reciprocal_sqrt fuses sqrt + reciprocal in one ACT op (and
        # lives in the same act-table set as Square/Copy -> no reloads).
        rcp = stat_sb.tile([1, TOK], f32r, tag="st_rcp")
        nc.scalar.activation(rcp[:], nsq_ps[:], AF.Abs_reciprocal_sqrt,
                             scale=scale)
        return rcp

    # =====================================================
    # P1+P3: per-(batch,head-group) QKV + attention, all local
    # =====================================================
    with tc.tile_pool(name="qkv_sb", bufs=1) as qkv_sb:
        qh_t = [qkv_sb.tile([D, T], bf16, tag=f"qh{u}", name=f"qh{u}")
                for u in range(4)]
        kh_t = [qkv_sb.tile([D, T], bf16, tag=f"kh{u}", name=f"kh{u}")
                for u in range(4)]
        vloc8 = [qkv_sb.tile([128, 2, 4 * D], f8, tag=f"vl{tb2}",
                             name=f"vl{tb2}") for tb2 in range(K2)]

        with tc.tile_pool(name="p1_hb", bufs=1) as p1hb, \
             tc.tile_pool(name="p1_w", bufs=2) as p1w, \
             tc.tile_pool(name="p1_tmp", bufs=2) as p1t, \
             tc.tile_pool(name="p1_cos", bufs=1) as p1cos:

            # dma_start costs ~1.3us of issue time on its sequencer, so
            # batch aggressively. First hb8 pair + q-weight strip go on the
            # scalar queue so they beat the bulk stream; phase A starts on
            # hb8 k2=0 + wq0.
            hb8a = p1hb.tile([128, K2, 2, T], f8, tag="hb8", name="hb8")
            hb8 = [hb8a[:, k2] for k2 in range(K2)]
            wq0 = p1w.tile([128, K2, 2, D], f8, tag="wq0", name="wq0", bufs=1)
            nc.scalar.dma_start(out=wq0[:], in_=E["wq_my"][0:128, :])
            nc.scalar.dma_start(out=hb8a[:, 0:2], in_=E["hb8_t"][:, 0:4 * T])
            cos_g = p1cos.tile([D, T], bf16, tag="cosg", name="cosg")
            sneg_g = p1cos.tile([D, T], bf16, tag="snegg", name="snegg")
            nc.scalar.dma_start(out=cos_g[:], in_=E["cos_g"][:])
            nc.scalar.dma_start(out=sneg_g[:], in_=E["sneg_g"][:])

            for j in range(1, 4):
                nc.sync.dma_start(
                    out=hb8a[:, 2 * j:2 * j + 2],
                    in_=E["hb8_t"][:, j * 4 * T:(j + 1) * 4 * T])
            # all k/q weight strips: one DMA each (partition-major views)
            wkall = p1w.tile([128, 4, K2, 2, D], f8, tag="wkall",
                             name="wkall", bufs=1)
            nc.sync.dma_start(
                out=wkall[:],
                in_=E["wk_my"].rearrange("(u p) c -> p u c", p=128))
            wqrest = p1w.tile([128, 3, K2, 2, D], f8, tag="wqrest",
                              name="wqrest", bufs=1)
            nc.sync.dma_start(
                out=wqrest[:],
                in_=E["wq_my"].rearrange("(u p) c -> p u c", p=128)[:, 1:4])

            with tc.tile_pool(name="p1_qkps", bufs=4, space="PSUM") as p1qkps, \
                 tc.tile_pool(name="p1_stps", bufs=2, space="PSUM") as p1stps:

                def qk_epilogue(ps, dst, u, tc4):
                    """rope + justnorm + sqk on one [D, 512] psum chunk."""
                    cs = (slice(0, D), slice(512 * tc4, 512 * (tc4 + 1)))
                    t1 = p1t.tile([D, 512], bf16, tag="ropet1", name="ropet1")
                    nc.vector.tensor_mul(t1[:], ps[:], cos_g[cs])
                    t2 = p1t.tile([D, 512], bf16, tag="ropet2", name="ropet2")
                    nc.vector.tensor_mul(
                        t2[0:64, :], ps[64:128, :],
                        sneg_g[0:64, 512 * tc4:512 * (tc4 + 1)])
                    nc.vector.tensor_mul(
                        t2[64:128, :], ps[0:64, :],
                        sneg_g[64:128, 512 * tc4:512 * (tc4 + 1)])
                    qp = p1t.tile([D, 512], bf16, tag="ropeqp", name="ropeqp")
                    nc.vector.tensor_add(qp[:], t1[:], t2[:])
                    sq = p1t.tile([D, 512], bf16, tag="ropesq", name="ropesq")
                    nc.scalar.activation(sq[:], qp[:], AF.Square)
                    nsq = p1stps.tile([1, 512], f32, tag="nsq", name="nsq")
                    nc.tensor.matmul(nsq[:], ones_col_b[:], sq[:],
                                     start=True, stop=True)
                    rcp = p1t.tile([1, 512], f32r, tag="rcp", name="rcp")
                    nc.scalar.activation(rcp[:], nsq[:],
                                         AF.Abs_reciprocal_sqrt)
                    rb = p1stps.tile([D, 512], f32, tag="rb", name="rb")
                    nc.tensor.matmul(rb[:], ones_row[:], rcp[:],
                                     start=True, stop=True)
                    nc.vector.scalar_tensor_tensor(
                        dst[u][cs], in0=qp[:], scalar=sqk_t[:, u:u + 1],
                        in1=rb[:], op0=ALU.mult, op1=ALU.mult)

                def qk_tap(dst, u, tapname):
                    if tapname in taps:
                        qf = p1t.tile([D, T], f32, tag="qtapf", name="qtapf")
                        nc.vector.tensor_copy(qf[:], dst[u][:])
                        nc.sync.dma_start(
                            out=taps[tapname][128 * u:128 * (u + 1), :],
                            in_=qf[:])

                # ---- phase A: q u=0, k2-outer so PE starts on hb8[0] ----
                psA = [p1qkps.tile([D, 512], f32, tag="qkps", name="qkps")
                       for _ in range(4)]
                for k2 in range(K2):
                    for tc4 in range(4):
                        nc.tensor.matmul(
                            psA[tc4][:], wq0[:, k2, :, :],
                            hb8[k2][:, :, 512 * tc4:512 * (tc4 + 1)],
                            start=(k2 == 0), stop=(k2 == K2 - 1),
                            perf_mode=DR)
                for tc4 in range(4):
                    qk_epilogue(psA[tc4], qh_t, 0, tc4)
                qk_tap(qh_t, 0, "qhat")

                # ---- phase B: k u=0..3, q u=1..3, tc4-outer ----
                def qk_proj(wall, uoff, dst, tapname, u_range):
                    for u in u_range:
                        for tc4 in range(4):
                            ps = p1qkps.tile([D, 512], f32, tag="qkps",
                                             name="qkps")
                            for k2 in range(K2):
                                nc.tensor.matmul(
                                    ps[:], wall[:, u - uoff, k2, :, :],
                                    hb8[k2][:, :, 512 * tc4:512 * (tc4 + 1)],
                                    start=(k2 == 0), stop=(k2 == K2 - 1),
                                    perf_mode=DR)
                            qk_epilogue(ps, dst, u, tc4)
                        qk_tap(dst, u, tapname)

                qk_proj(wkall, 0, kh_t, "khat", range(4))
                qk_proj(wqrest, 1, qh_t, "qhat", range(1, 4))

            # ---- v: token-major [tok, 4D] for the whole batch ----
            wv_res = p1w.tile([128, K2, 2, 4 * D], f8, tag="wvres",
                              name="wvres", bufs=1)
            nc.sync.dma_start(out=wv_res[:], in_=E["wv_my"][:])
            with tc.tile_pool(name="p1_vps", bufs=4, space="PSUM") as p1vps:
                for tb in range(KB):
                    vp = p1vps.tile([128, 4 * D], f32, tag="vp", name="vp")
                    for k2 in range(K2):
                        nc.tensor.matmul(
                            vp[:], hb8[k2][:, :, 128 * tb:128 * (tb + 1)],
                            wv_res[:, k2, :, :], start=(k2 == 0),
                            stop=(k2 == K2 - 1), perf_mode=DR)
                    nc.vector.tensor_copy(vloc8[tb // 2][:, tb % 2, :], vp[:])
                    if "vtok" in taps:
                        vf = p1t.tile([128, 4 * D], f32, tag="vtapf",
                                      name="vtapf")
                        nc.vector.tensor_copy(vf[:], vp[:])
                        nc.sync.dma_start(
                            out=taps["vtok"][128 * tb:128 * (tb + 1), :],
                            in_=vf[:])

        if PHASE_LEVEL <= 1:
            return

        # y_all viewed as [p, row-block, t] for the gathered reads
        yall1_v = E["y_all1"].rearrange("(rb p) t -> p rb t", p=128)
        yall2_v = E["y_all2"].rearrange("(rb p) t -> p rb t", p=128)

        # non-nested lifetimes (attention-start .. end of P4 matmuls) ->
        # manual alloc/release, created in space freed by the P1 pools
        p4y = tc.alloc_tile_pool(name="p4_y", bufs=1, side="right")
        p4wo = tc.alloc_tile_pool(name="p4_wo", bufs=1, side="right")
        yA1 = p4y.tile([128, K2, TOK], f8, tag="yA1", name="yA1")
        yA2 = p4y.tile([128, K2, TOK], f8, tag="yA2", name="yA2")
        wo8 = p4wo.tile([128, 2, K2, 2, 1024], f8, tag="wo8", name="wo8")
        # Wo fp8 block: bulk 4MB load, runs during attention (finishes
        # long before P4 consumes it).
        nc.sync.dma_start(out=wo8[:], in_=E["wo8_t"][:])

        # ---- attention: fully SBUF-local ----
        with tc.tile_pool(name="att_sb", bufs=6) as att_sb, \
             tc.tile_pool(name="att_y", bufs=1) as att_y, \
             tc.tile_pool(name="att_sps", bufs=2, space="PSUM") as att_sps, \
             tc.tile_pool(name="att_yd", bufs=2, space="PSUM") as att_yd, \
             tc.tile_pool(name="att_rb", bufs=1, space="PSUM") as att_rb:
            for u in range(4):
                ybig = att_y.tile([D, T], f8, tag=f"ybig{u % 2}",
                                  name=f"ybig{u % 2}")
                for t in range(4):
                    yps = att_yd.tile([D, 512], f32, tag="yps", name="yps")
                    dps = att_yd.tile([1, 512], f32, tag="dps", name="dps",
                                      bufs=1)
                    nblk2 = 2 * (t + 1)
                    for kb2 in range(nblk2):
                        sps2 = att_sps.tile([128, 1024], f32, tag="sps",
                                            name="sps")
                        for j in range(2):
                            kb = 2 * kb2 + j
                            nc.tensor.matmul(
                                sps2[:, 512 * j:512 * (j + 1)],
                                kh_t[u][:, 128 * kb:128 * (kb + 1)],
                                qh_t[u][:, 512 * t:512 * (t + 1)],
                                start=True, stop=True)
                        pT8 = att_sb.tile([128, 2, 512], f8e5, tag="pT",
                                          name="pT")
                        nc.scalar.activation(
                            pT8[:].rearrange("p a b -> p (a b)"), sps2[:],
                            AF.Exp, scale=SOFTMAX_SCALE, bias=ebias_t)
                        if 2 * kb2 + 1 >= 4 * t:
                            for j in range(2):
                                kb = 2 * kb2 + j
                                nc.gpsimd.affine_select(
                                    pT8[:, j, :], pT8[:, j, :],
                                    pattern=[[1, 512]],
                                    compare_op=ALU.is_ge, fill=0.0,
                                    base=512 * t - 128 * kb,
                                    channel_multiplier=-1)
                        nc.tensor.matmul(dps[:], ones8e5[:, :, 0:1], pT8[:],
                                         start=(kb2 == 0),
                                         stop=(kb2 == nblk2 - 1),
                                         perf_mode=DR)
                        nc.tensor.matmul(
                            yps[:], vloc8[kb2][:, :, 128 * u:128 * (u + 1)],
                            pT8[:], start=(kb2 == 0),
                            stop=(kb2 == nblk2 - 1), perf_mode=DR)
                    # +1e-6 denominator guard (paranoia; e5m2 can't underflow
                    # a whole row in practice)
                    dsb = att_sb.tile([1, 512], f32, tag="dsb", name="dsb")
                    nc.scalar.activation(dsb[:], dps[:], AF.Copy, bias=1e-6)
                    rd = att_sb.tile([1, 512], f32r, tag="rd", name="rd")
                    nc.vector.reciprocal(rd[:], dsb[:])
                    rdb = att_rb.tile([128, 512], f32, tag="rdb", name="rdb")
                    nc.tensor.matmul(rdb[:], ones_row[:], rd[:],
                                     start=True, stop=True)
                    ysb = att_sb.tile([D, 512], f32, tag="ysb", name="ysb")
                    nc.vector.tensor_copy(ysb[:], yps[:])
                    nc.vector.tensor_mul(ybig[:, 512 * t:512 * (t + 1)],
                                         ysb[:], rdb[:])
                ym = E["y_mine1"] if u < 2 else E["y_mine2"]
                nc.sync.dma_start(
                    out=ym[128 * (u % 2):128 * (u % 2 + 1), :], in_=ybig[:])
                if "ymine" in taps:
                    yf = att_y.tile([D, T], f32, tag="ytapf", name="ytapf")
                    nc.vector.tensor_copy(yf[:], ybig[:])
                    nc.sync.dma_start(
                        out=taps["ymine"][128 * u:128 * (u + 1), :], in_=yf[:])
                if u == 1:
                    allgather(E["y_mine1"], E["y_all1"])
                    # pull my token slices of slab 1 while u=2,3 compute
                    nc.sync.dma_start(out=yA1[:, :, 0:SL],
                                      in_=yall1_v[:, 0:8, ds(PC_reg, SL)])
                    nc.sync.dma_start(out=yA1[:, :, SL:2 * SL],
                                      in_=yall1_v[:, 8:16, ds(PC1_reg, SL)])
            allgather(E["y_mine2"], E["y_all2"])
            nc.sync.dma_start(out=yA2[:, :, 0:SL],
                              in_=yall2_v[:, 0:8, ds(PC_reg, SL)])
            nc.sync.dma_start(out=yA2[:, :, SL:2 * SL],
                              in_=yall2_v[:, 8:16, ds(PC1_reg, SL)])

    # =====================================================
    # P4+P5: Wo (fp8 DoubleRow), jn stats, residual 1 -> h2
    # =====================================================
    with tc.tile_pool(name="p4_sb", bufs=1) as p4sb, \
         tc.tile_pool(name="p4_tmp", bufs=2) as p4t:
        ha = [p4sb.tile([128, TOK], bf16, tag=f"ha{k}", name=f"ha{k}")
              for k in range(KB)]
        sqa = [p4sb.tile([128, TOK], bf16, tag=f"sqa{k}", name=f"sqa{k}")
               for k in range(KB)]
        u1a = [p4sb.tile([128, TOK], f32, tag=f"u1a{k}", name=f"u1a{k}")
               for k in range(KB)]

        with tc.tile_pool(name="hT_pool", bufs=1) as hT_pool:
            # load hT + jn(h) stats (fills the AllGather wait)
            hTa = hT_pool.tile([128, KB, TOK], f32r, tag="hT", name="hT")
            hT = [hTa[:, k] for k in range(KB)]
            # scalar queue + 4 chunks: must not head-block the sync queue's
            # yA2 loads right after the AllGather
            hT_v = E["h_t"].rearrange("(k p) t -> p k t", p=128)
            for j in range(4):
                nc.scalar.dma_start(out=hTa[:, 4 * j:4 * (j + 1)],
                                    in_=hT_v[:, 4 * j:4 * (j + 1)])
            rbh_sb = hT_pool.tile([128, TOK], f32, tag="rbh_sb", name="rbh_sb")
            with tc.tile_pool(name="p2_tmp", bufs=2) as p2t, \
                 tc.tile_pool(name="p2_stps", bufs=1, space="PSUM") as hstps:
                nsq_h = hstps.tile([1, TOK], f32, tag="nsq_h", name="nsq_h")
                for k in range(KB):
                    sq = p2t.tile([128, TOK], f32r, tag="hsq", name="hsq")
                    nc.vector.tensor_mul(sq[:], hT[k][:], hT[k][:])
                    nc.tensor.matmul(nsq_h[:], ones_col[:], sq[:],
                                     start=(k == 0), stop=(k == KB - 1))
                rcp_h = stats_from_psum(nsq_h, "h")
                rbh = hstps.tile([128, TOK], f32, tag="rbh", name="rbh")
                nc.tensor.matmul(rbh[:], ones_row[:], rcp_h[:],
                                 start=True, stop=True)
                nc.vector.tensor_copy(rbh_sb[:], rbh[:])

            if PHASE_LEVEL <= 2:
                p4wo.release()
                p4y.release()
                return

            with tc.tile_pool(name="p4_ps", bufs=2, space="PSUM") as p4ps:
                # even k2 (slab 1, AllGather-1) first, and paired groups:
                # both groups' yA1-only matmuls run before anything touches
                # yA2, widening the AllGather-2 latency cover. Per-psum
                # accumulation order is unchanged (evens then odds).
                k2_ev = [0, 2, 4, 6]
                k2_od = [1, 3, 5, 7]

                def p4_mms(pss, grp, k2s, start, stop):
                    for ki, k2 in enumerate(k2s):
                        slab = yA1 if k2 % 2 == 0 else yA2
                        rsrc = slab[:, 2 * (k2 // 2):2 * (k2 // 2) + 2, :]
                        for i in range(4):
                            f = 4 * grp + i
                            fh, x0 = f // 8, (f % 8) * 128
                            nc.tensor.matmul(
                                pss[i][:], wo8[:, fh, k2, :, x0:x0 + 128],
                                rsrc, start=(start and ki == 0),
                                stop=(stop and ki == len(k2s) - 1),
                                perf_mode=DR)

                def p4_drain(pss, grp):
                    for i in range(4):
                        f = 4 * grp + i
                        nc.vector.tensor_copy(ha[f][:], pss[i][:])
                        nc.scalar.activation(sqa[f][:], ha[f][:], AF.Square)
                        # res1 u1 = (1-lr_a) * jn(h), off the critical path
                        nc.vector.scalar_tensor_tensor(
                            u1a[f][:], in0=hT[f][:],
                            scalar=alr1_t[:, f:f + 1], in1=rbh_sb[:],
                            op0=ALU.mult, op1=ALU.mult)
                        if "hatt" in taps:
                            hf = p4t.tile([128, TOK], f32, tag="hatapf",
                                          name="hatapf")
                            nc.vector.tensor_copy(hf[:], ha[f][:])
                            nc.sync.dma_start(
                                out=taps["hatt"][128 * f:128 * (f + 1), :],
                                in_=hf[:])

                for gp in range(2):
                    ga, gb = 2 * gp, 2 * gp + 1
                    pssa = [p4ps.tile([128, TOK], f32, tag=f"wops{i}",
                                      name=f"wops{i}") for i in range(4)]
                    pssb = [p4ps.tile([128, TOK], f32, tag=f"wops{i}",
                                      name=f"wops{i}") for i in range(4)]
                    p4_mms(pssa, ga, k2_ev, start=True, stop=False)
                    p4_mms(pssb, gb, k2_ev, start=True, stop=False)
                    p4_mms(pssa, ga, k2_od, start=False, stop=True)
                    p4_drain(pssa, ga)
                    p4_mms(pssb, gb, k2_od, start=False, stop=True)
                    p4_drain(pssb, gb)
        p4wo.release()
        p4y.release()

        h2_pool = tc.alloc_tile_pool(name="h2_pool", bufs=1, side="right")
        h2 = [h2_pool.tile([128, TOK], f32r, tag=f"h2_{k}", name=f"h2_{k}")
              for k in range(KB)]

        with tc.tile_pool(name="p4_stps", bufs=1, space="PSUM") as p4stps:
            nsq_a = p4stps.tile([1, TOK], f32, tag="nsq_a", name="nsq_a")
            for k in range(KB):
                nc.tensor.matmul(nsq_a[:], ones_col_b[:], sqa[k][:],
                                 start=(k == 0), stop=(k == KB - 1))
            rcp_a = stats_from_psum(nsq_a, "a")

        with tc.tile_pool(name="r1_g", bufs=1) as r1g:
            _residual(tc, p4t, r1g, cbits, u1a, ha, rcp_a, alr_t,
                      out_r=h2, out_dram=taps.get("h2"), tagp="r1",
                      sqrt_scale=1.0 / (H2_SCALE * H2_SCALE))

    if PHASE_LEVEL <= 3:
        h2_pool.release()
        return

    # =====================================================
    # P6+P7: MLP, fp8 DoubleRow (h2 is 32*justnorm(h2') by construction)
    # =====================================================
    with tc.tile_pool(name="p7_sb", bufs=1) as p7sb, \
         tc.tile_pool(name="mlp_tmp", bufs=2) as mlpt:
        hm = [p7sb.tile([128, TOK], bf16, tag=f"hm{k}", name=f"hm{k}")
              for k in range(KB)]
        sqm = [p7sb.tile([128, TOK], bf16, tag=f"sqm{k}", name=f"sqm{k}")
               for k in range(KB)]
        u1m = [p7sb.tile([128, TOK], f32, tag=f"u1m{k}", name=f"u1m{k}")
               for k in range(KB)]
        with tc.tile_pool(name="p6_xm", bufs=1) as p6xm, \
             tc.tile_pool(name="p6_tmp", bufs=2) as p6t:

            xm8 = [p6xm.tile([128, 2, TOK], f8, tag=f"xm{j}", name=f"xm{j}")
                   for j in range(JG)]
            p7w = tc.alloc_tile_pool(name="p7_wd", bufs=2)
            wstrip0 = None
            with tc.tile_pool(name="p6_h8", bufs=1) as p6h8, \
                 tc.tile_pool(name="p6_wu", bufs=2) as p6w, \
                 tc.tile_pool(name="p6_ps", bufs=2, space="PSUM") as p6ps:
                h28 = [p6h8.tile([128, 2, TOK], f8, tag=f"h28_{k2}",
                                 name=f"h28_{k2}") for k2 in range(K2)]
                for k2 in range(K2):
                    for j in range(2):
                        nc.scalar.activation(
                            h28[k2][:, j, :],
                            h2[2 * k2 + j][:].bitcast(f32), AF.Copy)

                for jg in range(JG):
                    if jg == 24:
                        # prefetch the first down-proj strip behind the
                        # remaining up-proj strips
                        wstrip0 = p7w.tile([128, 4, 2, 1024], f8,
                                           tag="wpstrip", name="wpstrip")
                        nc.sync.dma_start(out=wstrip0[:],
                                          in_=E["wproj8_t"][:, 0:8192])
                    wt = p6w.tile([128, K2, 2, 512], f8, tag="wfct",
                                  name="wfct")
                    nc.sync.dma_start(
                        out=wt[:],
                        in_=E["wfc8_t"][:, jg * 8192:(jg + 1) * 8192])
                    ups = [p6ps.tile([128, TOK], f32, tag=f"ups{i}",
                                     name=f"ups{i}") for i in range(2)]
                    vps = [p6ps.tile([128, TOK], f32, tag=f"vps{i}",
                                     name=f"vps{i}") for i in range(2)]
                    for k2 in range(K2):
                        for i in range(2):
                            nc.tensor.matmul(
                                ups[i][:], wt[:, k2, :, 128 * i:128 * (i + 1)],
                                h28[k2][:], start=(k2 == 0),
                                stop=(k2 == K2 - 1), perf_mode=DR)
                            nc.tensor.matmul(
                                vps[i][:],
                                wt[:, k2, :, 256 + 128 * i:256 + 128 * (i + 1)],
                                h28[k2][:], start=(k2 == 0),
                                stop=(k2 == K2 - 1), perf_mode=DR)
                    for i in range(2):
                        sil = p6t.tile([128, TOK], bf16, tag="sil",
                                       name="sil")
                        nc.scalar.activation(sil[:], vps[i][:], AF.Silu,
                                             scale=1.0 / H2_SCALE)
                        nc.vector.scalar_tensor_tensor(
                            xm8[jg][:, i, :], in0=ups[i][:], scalar=c132_t,
                            in1=sil[:], op0=ALU.mult, op1=ALU.mult)

            # ---- MLP down (fp8 DoubleRow) ----
            # res2 u1 = (1-lr_m)*jn(h2) = h2*mlr1 (mlr1 absorbs the /32)
            for k in range(KB):
                nc.vector.tensor_scalar_mul(u1m[k][:], h2[k][:],
                                            mlr1_t[:, k:k + 1])
            with tc.tile_pool(name="p7_ps", bufs=1, space="PSUM") as p7ps:
                for fh in range(2):
                    pss = [p7ps.tile([128, TOK], f32, tag=f"wpps{i}",
                                     name=f"wpps{i}") for i in range(8)]
                    for j4 in range(8):
                        if fh == 0 and j4 == 0:
                            wstrip = wstrip0
                        else:
                            wstrip = p7w.tile([128, 4, 2, 1024], f8,
                                              tag="wpstrip", name="wpstrip")
                            nc.sync.dma_start(
                                out=wstrip[:],
                                in_=E["wproj8_t"][
                                    :, (fh * 8 + j4) * 8192:
                                    (fh * 8 + j4 + 1) * 8192])
                        if fh == 1 and j4 == 0:
                            # i-outer: each psum slot is touched only after
                            # fh=0's copy of that slot, hiding the copy tail
                            for i in range(8):
                                for jj in range(4):
                                    nc.tensor.matmul(
                                        pss[i][:],
                                        wstrip[:, jj, :,
                                               128 * i:128 * (i + 1)],
                                        xm8[4 * j4 + jj][:], start=(jj == 0),
                                        stop=False, perf_mode=DR)
                        else:
                            for jj in range(4):
                                j2 = 4 * j4 + jj
                                for i in range(8):
                                    nc.tensor.matmul(
                                        pss[i][:],
                                        wstrip[:, jj, :, 128 * i:128 * (i + 1)],
                                        xm8[j2][:], start=(fh == 0 and j2 == 0),
                                        stop=(j2 == JG - 1), perf_mode=DR)
                    for i in range(8):
                        f = 8 * fh + i
                        nc.vector.tensor_copy(hm[f][:], pss[i][:])
                        nc.scalar.activation(sqm[f][:], hm[f][:], AF.Square)
                        if "hmlp" in taps:
                            hf = p6t.tile([128, TOK], f32, tag="hmtapf",
                                          name="hmtapf")
                            nc.vector.tensor_copy(hf[:], hm[f][:])
                            nc.sync.dma_start(
                                out=taps["hmlp"][128 * f:128 * (f + 1), :],
                                in_=hf[:])

            p7w.release()
            with tc.tile_pool(name="p7_stps", bufs=1,
                              space="PSUM") as p7stps:
                nsq_m = p7stps.tile([1, TOK], f32, tag="nsq_m",
                                    name="nsq_m")
                for k in range(KB):
                    nc.tensor.matmul(nsq_m[:], ones_col_b[:], sqm[k][:],
                                     start=(k == 0), stop=(k == KB - 1))
                rcp_m = stats_from_psum(nsq_m, "m")

        # residual 2 -> output (xm freed)
        with tc.tile_pool(name="r2_g", bufs=1) as r2g:
            _residual(tc, mlpt, r2g, cbits, u1m, hm, rcp_m, mlr_t,
                      out_r=None, out_dram=E["out_t"], tagp="r2",
                      sqrt_scale=1.0)
    h2_pool.release()


def _residual(tc, tmp_pool, g_pool, cbits, u1_tiles, br_tiles, rcp_br,
              lr_tile, out_r, out_dram, tagp, sqrt_scale):
    """out = justnorm(u1 + lr (.) jn(br)), feature-major.

    u1_tiles hold the precomputed (1-lr) (.) jn(base) term. With
    sqrt_scale = 1/s^2 the output is s * justnorm(g) (used to store
    h2 pre-scaled for its fp8 cast).
    """
    nc = tc.nc
    ones_col, ones_row = cbits["ones_col"], cbits["ones_row"]

    with tc.tile_pool(name=tagp + "_ps", bufs=1, space="PSUM") as ps, \
         tc.tile_pool(name=tagp + "_sps", bufs=1, space="PSUM") as sps_pool:
        rba = ps.tile([128, TOK], f32, tag="rba", name="rba")
        nc.tensor.matmul(rba[:], ones_row[:], rcp_br[:], start=True, stop=True)
        nsq_g = sps_pool.tile([1, TOK], f32, tag="nsq_g", name="nsq_g")
        g = [g_pool.tile([128, TOK], f32, tag=f"g{k}", name=f"g{k}")
             for k in range(KB)]
        for k in range(KB):
            u2 = tmp_pool.tile([128, TOK], f32, tag="res_u2", name="res_u2")
            nc.vector.scalar_tensor_tensor(
                u2[:], in0=br_tiles[k][:], scalar=lr_tile[:, k:k + 1],
                in1=rba[:], op0=ALU.mult, op1=ALU.mult)
            nc.vector.tensor_add(g[k][:], u1_tiles[k][:], u2[:])
            sq = tmp_pool.tile([128, TOK], f32r, tag="res_sq", name="res_sq")
            nc.scalar.activation(sq[:], g[k][:], AF.Square)
            nc.tensor.matmul(nsq_g[:], ones_col[:], sq[:],
                             start=(k == 0), stop=(k == KB - 1))
        rcp_g = tmp_pool.tile([1, TOK], f32r, tag="res_rcp", name="res_rcp")
        nc.scalar.activation(rcp_g[:], nsq_g[:], AF.Abs_reciprocal_sqrt,
                             scale=sqrt_scale)
        rbg = ps.tile([128, TOK], f32, tag="rbg", name="rbg")
        nc.tensor.matmul(rbg[:], ones_row[:], rcp_g[:], start=True, stop=True)
        if out_r is not None:
            # evacuate rbg to SBUF so the PSUM pools release before the
            # output muls finish (the next phase needs all 8 banks)
            rbg_sb = g_pool.tile([128, TOK], f32, tag="rbg_sb",
                                 name="rbg_sb")
            nc.vector.tensor_copy(rbg_sb[:], rbg[:])
            rbg = rbg_sb
        if out_r is not None:
            for k in range(KB):
                nc.vector.tensor_mul(out_r[k][:], g[k][:], rbg[:])
                if out_dram is not None:
                    of = tmp_pool.tile([128, TOK], f32, tag="res_of",
                                       name="res_of")
                    nc.vector.tensor_copy(of[:], out_r[k][:].bitcast(f32))
                    nc.sync.dma_start(out=out_dram[128 * k:128 * (k + 1), :],
                                      in_=of[:])
        elif out_dram is not None:
            # group output writes 2 blocks per DMA: dma_start costs ~1.3us
            # of issue time, and 16 serial issues would pace this tail
            od_v = out_dram.rearrange("(k p) t -> p k t", p=128)
            for k2g in range(KB // 2):
                of = tmp_pool.tile([128, 2, TOK], f32, tag="res_of",
                                   name="res_of", bufs=3)
                for kk in range(2):
                    k = 2 * k2g + kk
                    nc.vector.tensor_mul(of[:, kk, :], g[k][:], rbg[:])
                nc.sync.dma_start(out=od_v[:, 2 * k2g:2 * (k2g + 1), :],
                                  in_=of[:])


# ============================================================
# host side
# ============================================================

_PROGRAM_CACHE = {}


def _get_program():
    key = (DEBUG_TAPS, PHASE_LEVEL, SIM_NO_CC)
    if key not in _PROGRAM_CACHE:
        _PROGRAM_CACHE[key] = _build_program()
    return _PROGRAM_CACHE[key]


def _to_f8(x):
    return np.clip(x, -240.0, 240.0).astype(mybir.dt.np(mybir.dt.float8e4))


def _host_prep(h, Wq, Wk, Wv, Wo, Wfc, Wproj, sqk, suv, attn_alpha, mlp_alpha):
    colmap = _rope_colmap()
    b16 = ml_dtypes.bfloat16
    wq_p = np.asarray(Wq)[:, colmap] * QKV_SCALE
    wk_p = np.asarray(Wk)[:, colmap] * QKV_SCALE
    wv_s = np.asarray(Wv) * QKV_SCALE
    sqk_p = (sqk * SQK_MULT)[colmap].astype(np.float32)

    # --- fp8 DoubleRow shared weights (see _emit for the index math) ---
    # wo8_t[p, fh, k2, j, x] = WO_SCALE * Wo[128*(2*k2+j)+p, 1024*fh+x]
    wo6 = (WO_SCALE * np.asarray(Wo)).reshape(K2, 2, 128, 2, 1024)
    wo8 = _to_f8(np.ascontiguousarray(
        wo6.transpose(2, 3, 0, 1, 4)).reshape(128, 2 * K2 * 2 * 1024))
    # wfc8_t[p, jg, k2, j, x]: x<256 -> u cols jg*256+x ; x>=256 -> v
    wfc_s = np.asarray(Wfc) * (np.asarray(suv) * SUV_MULT)[None, :]
    u4 = wfc_s[:, :4 * C].reshape(KB, 128, JG, 256)
    v4 = wfc_s[:, 4 * C:].reshape(KB, 128, JG, 256)
    uv = np.concatenate([u4, v4], axis=3)             # [k, p, jg, 512]
    uv6 = uv.reshape(K2, 2, 128, JG, 512)             # [k2, j, p, jg, x]
    wfc8 = _to_f8(np.ascontiguousarray(
        uv6.transpose(2, 3, 0, 1, 4)).reshape(128, JG * K2 * 2 * 512))
    # wproj8_t[p, fh, j4, jj, j, x] = WP_SCALE * Wproj[128*(2*(4*j4+jj)+j)+p,
    #                                                  1024*fh+x]
    wp7 = (WP_SCALE * np.asarray(Wproj)).reshape(8, 4, 2, 128, 2, 1024)
    wproj8 = _to_f8(np.ascontiguousarray(
        wp7.transpose(3, 4, 0, 1, 2, 5)).reshape(128, 2 * 8 * 4 * 2 * 1024))

    lr_a = np.abs(attn_alpha * ALPHA_MULT).astype(np.float32)
    lr_m = np.abs(mlp_alpha * ALPHA_MULT).astype(np.float32)
    lrs = np.stack([lr_a.reshape(KB, 128).T, lr_m.reshape(KB, 128).T,
                    (1 - lr_a).reshape(KB, 128).T,
                    ((1 - lr_m) / H2_SCALE).reshape(KB, 128).T],
                   axis=1)  # [128, 4, KB]
    lrs = np.concatenate(
        [np.ascontiguousarray(lrs.reshape(128, 4 * KB)),
         np.full((128, 1), 1.0 / H2_SCALE, np.float32),
         np.full((128, 1), EXP_BIAS5, np.float32),
         np.full((128, 1), 1e-6, np.float32)], axis=1)
    shared = {
        "wo8_t": wo8, "wfc8_t": wfc8, "wproj8_t": wproj8,
        "lrs": np.ascontiguousarray(lrs),
        "onesc": np.ones((128, 128), np.float32),
        "onesb": np.ones((128, 1), b16),
        "ones8e5": np.ones((128, 32), mybir.dt.np(mybir.dt.float8e5)),
    }

    inv_freq = 1.0 / (10000.0 ** (np.arange(0, D, 2, dtype=np.float32) / D))
    pos_g = np.arange(T, dtype=np.float32)
    ang_g = inv_freq[:, None] * pos_g[None, :]
    shared["cos_g"] = np.concatenate(
        [np.cos(ang_g), np.cos(ang_g)], axis=0).astype(b16)
    shared["sneg_g"] = np.concatenate(
        [-np.sin(ang_g), np.sin(ang_g)], axis=0).astype(b16)

    # hb8_t[p, k2, j, t] = h[b].T[128*(2*k2+j)+p, t] in fp8
    hb8 = [_to_f8(np.ascontiguousarray(
        h[b].T.reshape(K2, 2, 128, T).transpose(2, 0, 1, 3)).reshape(
            128, K2 * 2 * T)) for b in range(B)]

    def tile_qk(w, hc0):
        # [4*128, K2*2*D]: row u*128+p, col k2*2D+j*D+d
        #   = QKV_SCALE * w[128*(2*k2+j)+p, hc0+u*128+d]
        w6 = w[:, hc0:hc0 + 4 * D].reshape(K2, 2, 128, 4, D)
        return _to_f8(np.ascontiguousarray(
            w6.transpose(3, 2, 0, 1, 4)).reshape(4 * 128, K2 * 2 * D))

    in_maps = []
    for c in range(NCORES):
        bb = c // 4
        hc0 = 4 * (c % 4) * D
        hslice = np.concatenate([
            h[0, SL * c:SL * (c + 1), :].T,
            h[1, SL * (7 - c):SL * (8 - c), :].T], axis=1)
        wv6 = wv_s[:, hc0:hc0 + 4 * D].reshape(K2, 2, 128, 4 * D)
        m = dict(shared)
        m["h_t"] = np.ascontiguousarray(hslice, dtype=np.float32)
        m["hb8_t"] = hb8[bb]
        m["wq_my"] = tile_qk(wq_p, hc0)
        m["wk_my"] = tile_qk(wk_p, hc0)
        m["wv_my"] = _to_f8(np.ascontiguousarray(
            wv6.transpose(2, 0, 1, 3)).reshape(128, K2 * 2 * 4 * D))
        m["sqk_my"] = np.ascontiguousarray(
            sqk_p[hc0:hc0 + 4 * D].reshape(4, D).T)
        in_maps.append(m)
    return in_maps


def _unshard(results, key="out_t"):
    out = np.empty((B, T, C), np.float32)
    for c in range(NCORES):
        ot = results[c][key]
        out[0, SL * c:SL * (c + 1), :] = ot[:, 0:SL].T
        out[1, SL * (7 - c):SL * (8 - c), :] = ot[:, SL:2 * SL].T
    return out


def kernel(h, mask, Wq, Wk, Wv, Wo, Wfc, Wproj, sqk, suv, attn_alpha, mlp_alpha):
    h = np.asarray(h, np.float32)
    args = [np.asarray(a, np.float32) for a in
            (Wq, Wk, Wv, Wo, Wfc, Wproj, sqk, suv, attn_alpha, mlp_alpha)]
    nc = _get_program()
    in_maps = _host_prep(h, *args)
    res = run_bass_kernel_spmd(nc, in_maps, core_ids=list(range(NCORES)))
    return _unshard(res.results)
